# revision 5
# baseline (speedup 1.0000x reference)
"""Trainium2 Bass kernel for nn_Attention_49168785605257.

Causal multi-head self-attention: B=2, N=4096, DIM=512, H=8, DH=64.
Reference applies dim_head**-0.5 scaling TWICE (folded here into Wq as 1/64).

Sharding: one head per NeuronCore (8 cores). Each core computes its head's
attention for BOTH batches (packed into partition halves 0-63 / 64-127) and
its partial output projection o_h = attn_out_h @ Wo[64h:64h+64, :].  The host
sums the 8 partials and adds the bias.

Device-side formulation (per core):
  - All tensors carried transposed ([feature, token]) so the contraction dim
    sits on SBUF partitions; the host pre-transposes x.
  - Flash-attention in S^T orientation: S^T[j,i] tiles come straight out of
    the PE; exp on ScalarE (PSUM->SBUF, [128,1024] groups covering both
    batches); causal masking by multiplying the exp output of the 4 diagonal
    j-blocks per i-chunk with precomputed 0/1 masks; A@V accumulated in PSUM
    with v augmented by a ones-column so row 64 collects the softmax
    denominators; 1/den via Ln then Exp(-x) (one ACT table set); denominator
    broadcast across partitions on GPSIMD; normalize late (softmax linearity).
"""

import os
import sys
from contextlib import ExitStack

import numpy as np

for _p in ("/opt/trn_rl_repo", "/root/.axon_site/_ro/trn_rl_repo"):
    if _p not in sys.path and os.path.isdir(_p):
        sys.path.append(_p)

import ml_dtypes  # noqa: E402

B, N, DIM, H, DH = 2, 4096, 512, 8, 64
N_CORES = 8
CH = 512            # i-chunk width (tokens)
JB = 128            # j-block width (tokens)

BF16 = "bfloat16"
F32 = "float32"


def build_attention_kernel(nc, NB: int):
    """Emit the per-core program. NB = tokens per batch (4096 full size)."""
    import concourse.mybir as mybir
    import concourse.tile as tile

    bf16 = mybir.dt.bfloat16
    f32 = mybir.dt.float32
    mult = mybir.AluOpType.mult
    Exp = mybir.ActivationFunctionType.Exp
    Ln = mybir.ActivationFunctionType.Ln

    NCH = NB // CH          # i-chunks per batch
    JTB = NB // JB          # j-blocks per batch

    xT_d = nc.dram_tensor("xT", [DIM, 2 * NB], bf16, kind="ExternalInput").ap()
    wq_d = nc.dram_tensor("wq", [DIM, DH], bf16, kind="ExternalInput").ap()
    wk_d = nc.dram_tensor("wk", [DIM, DH], bf16, kind="ExternalInput").ap()
    wv_d = nc.dram_tensor("wv", [DIM, DH], bf16, kind="ExternalInput").ap()
    wo_d = nc.dram_tensor("wo", [DH, DIM], bf16, kind="ExternalInput").ap()
    mask_d = nc.dram_tensor("masks", [512, 1024], bf16, kind="ExternalInput").ap()
    idup_d = nc.dram_tensor("identup", [128, DH], bf16, kind="ExternalInput").ap()
    oT_d = nc.dram_tensor("oT", [DIM, 2 * NB], f32, kind="ExternalOutput").ap()

    with tile.TileContext(nc) as tc, ExitStack() as ctx:
        const = ctx.enter_context(tc.tile_pool(name="const", bufs=1))
        xpool = ctx.enter_context(tc.tile_pool(name="xp", bufs=8))
        big = ctx.enter_context(tc.tile_pool(name="big", bufs=1))
        ptp = ctx.enter_context(tc.tile_pool(name="ptp", bufs=3))
        rp = ctx.enter_context(tc.tile_pool(name="rp", bufs=2))
        op_sb_pool = ctx.enter_context(tc.tile_pool(name="osb", bufs=4))
        ps_pool = ctx.enter_context(tc.tile_pool(name="ps", bufs=2, space="PSUM"))
        av_pool = ctx.enter_context(tc.tile_pool(name="av", bufs=1, space="PSUM"))
        op_pool = ctx.enter_context(tc.tile_pool(name="op", bufs=2, space="PSUM"))

        # ---- constants / weights ----
        wq_sb = const.tile([128, 4 * DH], bf16, tag="wq")
        wk_sb = const.tile([128, 4 * DH], bf16, tag="wk")
        wv_sb = const.tile([128, 4 * DH], bf16, tag="wv")
        for d in range(4):
            nc.sync.dma_start(wq_sb[:, d * DH:(d + 1) * DH], wq_d[128 * d:128 * (d + 1), :])
            nc.sync.dma_start(wk_sb[:, d * DH:(d + 1) * DH], wk_d[128 * d:128 * (d + 1), :])
            nc.sync.dma_start(wv_sb[:, d * DH:(d + 1) * DH], wv_d[128 * d:128 * (d + 1), :])
        wo_sb = const.tile([DH, DIM], bf16, tag="wo")
        nc.sync.dma_start(wo_sb[:], wo_d[:, :])
        mask_sb = const.tile([128, 4096], bf16, tag="mask")
        for t in range(4):
            nc.sync.dma_start(mask_sb[:, 1024 * t:1024 * (t + 1)], mask_d[128 * t:128 * (t + 1), :])
        idup_sb = const.tile([128, DH], bf16, tag="idup")
        nc.sync.dma_start(idup_sb[:], idup_d[:, :])

        # ---- persistent activations (partition halves: rows 0-63 batch0, 64-127 batch1) ----
        qT = big.tile([128, NB], bf16, tag="qT")
        kT = big.tile([128, NB], bf16, tag="kT")
        vT = big.tile([128, NB], bf16, tag="vT")
        vaug = [big.tile([128, 65 * JTB], bf16, tag=f"vaug{b}", name=f"vaug{b}")
                for b in range(2)]
        nc.gpsimd.memset(vaug[0][:], 1.0)
        nc.gpsimd.memset(vaug[1][:], 1.0)

        for c in range(NCH):
            i0 = CH * c
            # ---- load x chunk (both batches), project q/k/v ----
            xts = []
            for d in range(4):
                xt = xpool.tile([128, 1024], bf16, tag="xt")
                nc.sync.dma_start(xt[:, 0:512], xT_d[128 * d:128 * (d + 1), i0:i0 + CH])
                nc.sync.dma_start(xt[:, 512:1024], xT_d[128 * d:128 * (d + 1), NB + i0:NB + i0 + CH])
                xts.append(xt)
            for w_sb, dst in ((wq_sb, qT), (wk_sb, kT), (wv_sb, vT)):
                ps = ps_pool.tile([128, CH], f32, tag="s")
                for d in range(4):
                    nc.tensor.matmul(ps[0:64, :], w_sb[:, d * DH:(d + 1) * DH], xts[d][:, 0:512],
                                     start=(d == 0), stop=(d == 3), tile_position=(0, 0),
                                     skip_group_check=True)
                    nc.tensor.matmul(ps[64:128, :], w_sb[:, d * DH:(d + 1) * DH], xts[d][:, 512:1024],
                                     start=(d == 0), stop=(d == 3), tile_position=(0, 64),
                                     skip_group_check=True)
                nc.vector.tensor_copy(dst[:, i0:i0 + CH], ps[:, :])

            # ---- transpose v for the 4 new j-blocks (both batches via row groups) ----
            for tt in range(4 * c, 4 * c + 4):
                pst0 = ps_pool.tile([128, 64], bf16, tag="s", name="pst0")
                pst1 = ps_pool.tile([128, 64], bf16, tag="s", name="pst1")
                nc.tensor.matmul(pst0[:], vT[0:64, JB * tt:JB * (tt + 1)], idup_sb[0:64, :],
                                 is_transpose=True, tile_position=(0, 0), skip_group_check=True)
                nc.tensor.matmul(pst1[:], vT[64:128, JB * tt:JB * (tt + 1)], idup_sb[64:128, :],
                                 is_transpose=True, tile_position=(64, 0), skip_group_check=True)
                nc.vector.tensor_copy(vaug[0][:, 65 * tt:65 * tt + 64], pst0[:])
                nc.vector.tensor_copy(vaug[1][:, 65 * tt:65 * tt + 64], pst1[:])

            # ---- attention for i-chunk c ----
            pso = av_pool.tile([65, 1024], f32, tag="av")
            njb = 4 * (c + 1)
            for jb in range(njb):
                pss = ps_pool.tile([128, 1024], f32, tag="s")
                nc.tensor.matmul(pss[:, 0:512], kT[0:64, JB * jb:JB * (jb + 1)], qT[0:64, i0:i0 + CH],
                                 start=True, stop=True, tile_position=(0, 0), skip_group_check=True)
                nc.tensor.matmul(pss[:, 512:1024], kT[64:128, JB * jb:JB * (jb + 1)], qT[64:128, i0:i0 + CH],
                                 start=True, stop=True, tile_position=(64, 0), skip_group_check=True)
                pt = ptp.tile([128, 1024], bf16, tag="pt")
                nc.scalar.activation(pt[:], pss[:], Exp)
                if jb >= 4 * c:
                    t = jb - 4 * c
                    nc.vector.tensor_tensor(pt[:], pt[:], mask_sb[:, 1024 * t:1024 * (t + 1)], mult)
                nc.tensor.matmul(pso[:, 0:512], vaug[0][:, 65 * jb:65 * jb + 65], pt[:, 0:512],
                                 start=(jb == 0), stop=(jb == njb - 1), skip_group_check=True)
                nc.tensor.matmul(pso[:, 512:1024], vaug[1][:, 65 * jb:65 * jb + 65], pt[:, 512:1024],
                                 start=(jb == 0), stop=(jb == njb - 1), skip_group_check=True)

            # ---- epilogue: 1/den, normalize, output projection, stream out ----
            recip = rp.tile([65, 2048], f32, tag="recip")
            nc.scalar.activation(recip[64:65, 0:1024], pso[64:65, 0:1024], Ln)
            nc.scalar.activation(recip[64:65, 1024:2048], recip[64:65, 0:1024], Exp, scale=-1.0)
            riph = rp.tile([1, 1024], f32, tag="riph")
            nc.sync.dma_start(riph[0:1, :], recip[64:65, 1024:2048])
            recipb = rp.tile([64, 1024], f32, tag="recipb")
            nc.gpsimd.partition_broadcast(recipb[0:64, :], riph[0:1, :], channels=64)
            outTn = rp.tile([64, 1024], bf16, tag="outTn")
            nc.vector.tensor_tensor(outTn[:], pso[0:64, 0:1024], recipb[:], mult)
            for b in range(2):
                for dblk in range(4):
                    opp = op_pool.tile([128, 512], f32, tag="op")
                    nc.tensor.matmul(opp[:], wo_sb[:, 128 * dblk:128 * (dblk + 1)],
                                     outTn[0:64, 512 * b:512 * b + 512])
                    o_sb = op_sb_pool.tile([128, 512], f32, tag="o")
                    nc.vector.tensor_copy(o_sb[:], opp[:])
                    nc.sync.dma_start(oT_d[128 * dblk:128 * (dblk + 1), NB * b + i0:NB * b + i0 + CH],
                                      o_sb[:])
    return nc


def make_host_constants(NB: int):
    """Masks for the 4 diagonal j-block offsets and the stacked identity."""
    jj = np.arange(JB)[:, None]
    ii = np.arange(CH)[None, :]
    masks = np.zeros((512, 1024), np.float32)
    for t in range(4):
        m = (ii >= jj + JB * t).astype(np.float32)       # [128, 512]
        masks[128 * t:128 * (t + 1), :] = np.concatenate([m, m], axis=1)
    identup = np.concatenate([np.eye(DH, dtype=np.float32)] * 2, axis=0)  # [128, 64]
    return (masks.astype(ml_dtypes.bfloat16), identup.astype(ml_dtypes.bfloat16))


_CACHE = {}


def _get_compiled(NB: int):
    key = ("nc", NB)
    if key not in _CACHE:
        import concourse.bacc as bacc
        nc = bacc.Bacc("TRN2", debug=False, num_devices=N_CORES)
        build_attention_kernel(nc, NB)
        nc.compile()
        _CACHE[key] = nc
    return _CACHE[key]


def make_in_maps(x, Wq, Wkv, Wo, NB: int):
    bf = ml_dtypes.bfloat16
    nb_total = x.shape[0] * x.shape[1]
    xT = np.ascontiguousarray(x.reshape(nb_total, DIM).T).astype(bf)  # [512, B*NB]
    masks, identup = make_host_constants(NB)
    in_maps = []
    for h in range(N_CORES):
        s = slice(DH * h, DH * (h + 1))
        in_maps.append({
            "xT": xT,
            "wq": np.ascontiguousarray(Wq[:, s] / 64.0).astype(bf),
            "wk": np.ascontiguousarray(Wkv[:, DH * h:DH * (h + 1)]).astype(bf),
            "wv": np.ascontiguousarray(Wkv[:, DIM + DH * h:DIM + DH * (h + 1)]).astype(bf),
            "wo": np.ascontiguousarray(Wo[s, :]).astype(bf),
            "masks": masks,
            "identup": identup,
        })
    return in_maps


def kernel(x, Wq, Wkv, Wo, bo, _run_kwargs=None):
    from concourse.bass_utils import run_bass_kernel_spmd
    x = np.asarray(x, np.float32)
    NB = x.shape[1]
    nc = _get_compiled(NB)
    in_maps = make_in_maps(np.asarray(x), np.asarray(Wq), np.asarray(Wkv), np.asarray(Wo), NB)
    res = run_bass_kernel_spmd(nc, in_maps, core_ids=list(range(N_CORES)),
                               **(_run_kwargs or {}))
    oT = np.zeros((DIM, x.shape[0] * NB), np.float64)
    for c in range(N_CORES):
        oT += res.results[c]["oT"].astype(np.float64)
    out = oT.T.reshape(x.shape[0], NB, DIM).astype(np.float32) + np.asarray(bo, np.float32)
    if _run_kwargs is not None:
        _CACHE["last_results"] = res
    return out
